# revision 39
# baseline (speedup 1.0000x reference)
"""Trainium2 Bass kernel for nn_NatureCNN (CNN trunk + MLP + 2x2-layer LSTM + FC head).

Sharding: data-parallel over the segment axis. Core k owns segments
b in [4k, 4k+4) of both LSTM stacks, i.e. frames [128k,128k+128) (rnns
block A) and [1024+128k, +128) (rnn block B) -- 256 frames/core, fully
independent per core (zero communication). Host stacks the per-core
[4, 130] outputs.

On-chip layouts (channels-on-partitions):
  obsT   [48=(ic,ry,rx), 256f, (by,bx)=256]  phase-decomposed 4x4 blocks, bf16
  conv1  K=48, PSUM-accumulate over (dy,dx) block shifts; 4 col-tiles by
         output-row group j (rows {j, j+2, .., j+10}) -> R [(j,oc1)=128, ...]
  conv2  K=128=(ky2,ic2) directly on R; accumulate kx2 -> C2 [64, (f,6,6)]
  conv3  K=64; accumulate (ky3,kx3); 2 col-tiles by py3-pair -> C3
  MLP    dataT [1536, f] bf16 lin1; fp32r lin2 -> doutT
  LSTM   layer-skewed steps; gate order (f,i,g,o); gates layer0 at PSUM
         rows 0:4, layer1 at 32:36; xw contribution pre-accumulated into
         next step's PSUM via i4 extract while the cell phase runs;
         tanh split (f,i,g) on-chain / (o) off-chain;
         sigmoid(x)=(tanh(x/2)+1)/2; c stored as c/2 ("sC"), h stored
         as 2h (0.5 folded into weights); PE-transpose of h each step.
  head   feature-major fc1/fc2, batch on the moving free dim.

All weights are pre-laid out host-side in their exact SBUF layout so
every DMA is a contiguous [p, cols] copy (one large descriptor per
partition). Weight loads are issued on the scalar-engine DMA queue so
they don't head-of-line block the obs-block stream on the sync queue.
"""

import numpy as np
import ml_dtypes

import concourse.bass as bass
import concourse.mybir as mybir
import concourse.tile as tile
from concourse.bass_utils import run_bass_kernel_spmd

dt = mybir.dt
AF = mybir.ActivationFunctionType
ALU = mybir.AluOpType

N_CORES = 8
S, H = 32, 100
B_LOC = 4                          # segments per core
FPC = 2 * B_LOC * S                # 256 frames per core
FB = 32                            # frames per conv block
NBLK = FPC // FB
G4 = 4 * H                         # 400
DATA_DIM, RIN, OUT = 1500, 1536, 130
NSTEP = 2 * S + 1                  # 65 layer-skewed steps

bf = ml_dtypes.bfloat16

MAX_WAITS = 1
_ctr = [0]


def _fix_sync_waits(nc):
    """This env's walrus rejects >1 sem-wait per instruction; hoist extras
    onto same-engine NoOps inserted immediately before, preserving order."""
    for f in nc.m.functions:
        for bb in f.blocks:
            new_list = []
            for ins in bb.instructions:
                si = ins.sync_info
                if si is not None and si.on_wait and len(si.on_wait) > MAX_WAITS:
                    waits = list(si.on_wait)
                    for i in range(0, len(waits) - MAX_WAITS, MAX_WAITS):
                        _ctr[0] += 1
                        nop = mybir.InstNoOp(name=f"waitfix-{_ctr[0]}", ins=[], outs=[])
                        nop.engine = ins.engine
                        nop.sync_info = mybir.SyncInfo(
                            on_wait=waits[i : i + MAX_WAITS], on_update=[])
                        new_list.append(nop)
                        nc.register_instruction(nop, overwrite=True)
                    si.on_wait = waits[len(waits) - MAX_WAITS :]
                    ins.sync_info = si
                new_list.append(ins)
            bb.instructions = new_list


# --------------------------------------------------------------------------
# host-side weight/layout prep
# --------------------------------------------------------------------------

def _gate_perm():
    # torch gate order (i, f, g, o) -> ours (f, i, g, o)
    return np.concatenate([np.arange(100, 200), np.arange(0, 100),
                           np.arange(200, 300), np.arange(300, 400)])


def _feat_ch_order():
    """our featT row R -> original feat channel index."""
    orig = np.zeros(RIN, np.int64)
    for R in range(1024):
        c, rem = divmod(R, 128)
        g, oc3 = divmod(rem, 64)
        orig[R] = oc3 * 16 + (2 * g + c // 4) * 4 + (c % 4)
    orig[1024:] = np.arange(1024, RIN)
    return orig


def _prep_weights(inp):
    w = {}
    gp = _gate_perm()
    ch = _feat_ch_order()

    c1 = inp['conv1_w'].reshape(32, 3, 2, 4, 2, 4)   # oc, ic, dy, ry, dx, rx
    w1 = c1.transpose(2, 4, 1, 3, 5, 0).reshape(2, 2, 48, 32)  # dy, dx, p, oc
    # K=96 stationary: rows 0:48 dy=0 taps, rows 48:96 dy=1 taps; col block
    # per dx. Pairs with the row-shifted duplicate half of obsT.
    w1d = np.concatenate([w1[0], w1[1]], axis=1)     # dx, 96, 32
    w['w1'] = np.ascontiguousarray(w1d.transpose(1, 0, 2).reshape(96, 64)).astype(bf)
    w['cb1'] = np.tile(inp['conv1_b'], 4).reshape(128, 1).astype(np.float32)
    w2 = inp['conv2_w'].transpose(3, 2, 1, 0).reshape(4, 128, 64)
    w['w2'] = np.ascontiguousarray(w2.transpose(1, 0, 2).reshape(128, 256)).astype(bf)
    w['cb2'] = inp['conv2_b'].reshape(64, 1).astype(np.float32)
    w3 = inp['conv3_w'].transpose(2, 3, 1, 0).reshape(9, 64, 64)
    w['w3'] = np.ascontiguousarray(w3.transpose(1, 0, 2).reshape(64, 576)).astype(bf)
    w['cb3'] = np.tile(inp['conv3_b'], 2).reshape(128, 1).astype(np.float32)

    l1 = np.zeros((RIN, 1024), np.float32)
    l1[:DATA_DIM] = inp['lin1_w'].T
    w['lin1w'] = np.ascontiguousarray(
        l1.reshape(12, 128, 1024).transpose(1, 0, 2).reshape(128, 12 * 1024)).astype(bf)
    w['b1col'] = inp['lin1_b'].reshape(8, 128).T.copy().astype(np.float32)
    w['lin2w'] = np.ascontiguousarray(
        inp['lin2_w'].T.reshape(8, 128, 512).transpose(1, 0, 2)
        .reshape(128, 8 * 512)).astype(bf)
    w['b2col'] = inp['lin2_b'].reshape(4, 128).T.copy().astype(np.float32)

    def lstm_prep(pfx):
        # gate order (f,i,g,o); g rows (200:300) pre-scaled by 2 so a single
        # tanh(x/2) computes sigmoid-halves for f,i,o and tanh for g.
        wih0 = inp[f'{pfx}_wih0'][gp].copy()
        wih0[200:300] *= 2.0
        wih0T = wih0.T[ch].copy()                     # [1536, 400]
        b0 = (inp[f'{pfx}_bih0'] + inp[f'{pfx}_bhh0'])[gp].copy()
        b0[200:300] *= 2.0
        whh0 = inp[f'{pfx}_whh0'][gp].copy()
        whh0[200:300] *= 2.0
        wih1 = inp[f'{pfx}_wih1'][gp].copy()
        wih1[200:300] *= 2.0
        whh1 = inp[f'{pfx}_whh1'][gp].copy()
        whh1[200:300] *= 2.0
        b1 = (inp[f'{pfx}_bih1'] + inp[f'{pfx}_bhh1'])[gp].copy()
        b1[200:300] *= 2.0
        whh1ext = np.concatenate([whh1.T * 0.5, b1[None, :]], 0)
        return wih0T, b0, whh0.T * 0.5, wih1.T * 0.5, whh1ext

    a = [lstm_prep('rnns'), lstm_prep('rnn')]
    # SBUF layouts directly:
    wih0 = np.stack([x[0] for x in a]).reshape(2, 12, 128, G4)   # lst, kc, p, g
    w['wih0'] = np.ascontiguousarray(
        wih0.transpose(2, 0, 1, 3).reshape(128, 2 * 12 * G4)).astype(bf)
    w['b0row'] = np.stack([x[1] for x in a]).reshape(1, 2 * G4).astype(bf)
    w['whh0'] = np.ascontiguousarray(
        np.stack([x[2] for x in a]).transpose(1, 0, 2).reshape(100, 2 * G4)).astype(bf)
    w['wih1'] = np.ascontiguousarray(
        np.stack([x[3] for x in a]).transpose(1, 0, 2).reshape(100, 2 * G4)).astype(bf)
    w['whh1'] = np.ascontiguousarray(
        np.stack([x[4] for x in a]).transpose(1, 0, 2).reshape(101, 2 * G4)).astype(bf)

    fc1 = np.stack([inp['fc1_w'][:, :100].T * 0.5,
                    inp['fc1_w'][:, 100:].T * 0.5])              # [2, 100, 512]
    w['fc1w'] = np.ascontiguousarray(
        fc1.transpose(1, 0, 2).reshape(100, 1024)).astype(bf)
    w['fc1bcol'] = inp['fc1_b'].reshape(4, 128).T.copy().astype(np.float32)
    w['fc2w'] = np.ascontiguousarray(
        inp['fc2_w'].T.reshape(4, 128, OUT).transpose(1, 0, 2)
        .reshape(128, 4 * OUT)).astype(bf)
    w['fc2brow'] = inp['fc2_b'].reshape(1, OUT).astype(bf)

    i4 = np.zeros((128, 4), np.float32)
    for k in range(4):
        i4[32 * k : 32 * k + 4] = np.eye(4)
    w['i4rep'] = i4.astype(bf)
    w['ident36'] = np.eye(36).astype(bf)
    w['hT_init'] = np.concatenate(
        [np.zeros((100, 36), np.float32), np.ones((1, 36), np.float32)]).astype(bf)
    return w


def _prep_core_inputs(inp, w, k):
    idx = np.concatenate([np.arange(128 * k, 128 * k + 128),
                          np.arange(1024 + 128 * k, 1024 + 128 * k + 128)])
    obs = np.asarray(inp['observations'], np.float32)[idx]
    obsT = (obs.reshape(FPC, 3, 16, 4, 16, 4)
            .transpose(1, 3, 5, 0, 2, 4)
            .reshape(48, FPC, 16, 16)).astype(bf)
    # duplicate with block-rows shifted by one (the dy=1 phase) so conv1 can
    # contract K=96 in a single accumulation pair; block-major for 16KB descs
    sh = np.zeros_like(obsT)
    sh[:, :, :15, :] = obsT[:, :, 1:, :]
    obsT = np.concatenate([obsT, sh], axis=0).reshape(96, NBLK, FB * 256)
    obsT = np.ascontiguousarray(obsT.transpose(1, 0, 2))
    data = np.asarray(inp['data'], np.float32)[idx]
    dT = np.zeros((RIN, FPC), np.float32)
    dT[:DATA_DIM] = data.T
    dsb = np.ascontiguousarray(
        dT.reshape(12, 128, FPC).transpose(1, 0, 2).reshape(128, 12 * FPC)).astype(bf)
    m = {'obsT': obsT, 'dataT': dsb}
    m.update(w)
    return m


# --------------------------------------------------------------------------
# kernel IR
# --------------------------------------------------------------------------

def _build_nc():
    nc = bass.Bass("TRN2", target_bir_lowering=False, debug=False,
                   num_devices=N_CORES)

    D = {}
    def inp(name, shape, d):
        D[name] = nc.dram_tensor(name, shape, d, kind="ExternalInput")

    inp('obsT', [NBLK, 96, FB * 256], dt.bfloat16)
    inp('dataT', [128, 12 * FPC], dt.bfloat16)
    inp('w1', [96, 64], dt.bfloat16)
    inp('cb1', [128, 1], dt.float32)
    inp('w2', [128, 256], dt.bfloat16)
    inp('cb2', [64, 1], dt.float32)
    inp('w3', [64, 576], dt.bfloat16)
    inp('cb3', [128, 1], dt.float32)
    inp('lin1w', [128, 12 * 1024], dt.bfloat16)
    inp('b1col', [128, 8], dt.float32)
    inp('lin2w', [128, 8 * 512], dt.bfloat16)
    inp('b2col', [128, 4], dt.float32)
    inp('wih0', [128, 24 * G4], dt.bfloat16)
    inp('b0row', [1, 2 * G4], dt.bfloat16)
    inp('whh0', [100, 2 * G4], dt.bfloat16)
    inp('wih1', [100, 2 * G4], dt.bfloat16)
    inp('whh1', [101, 2 * G4], dt.bfloat16)
    inp('fc1w', [100, 2 * 512], dt.bfloat16)
    inp('fc1bcol', [128, 4], dt.float32)
    inp('fc2w', [128, 4 * OUT], dt.bfloat16)
    inp('fc2brow', [1, OUT], dt.bfloat16)
    inp('i4rep', [128, 4], dt.bfloat16)
    inp('ident36', [36, 36], dt.bfloat16)
    inp('hT_init', [101, 36], dt.bfloat16)

    out_d = nc.dram_tensor('out', [B_LOC, OUT], dt.float32, kind="ExternalOutput")
    xw_scr = nc.dram_tensor('xw_scr', [2, 128, G4], dt.bfloat16)  # internal scratch

    with tile.TileContext(nc) as tc:
        with (
            tc.tile_pool(name="const", bufs=1) as cpool,
            tc.tile_pool(name="acts", bufs=1) as apool,
            tc.tile_pool(name="conv", bufs=2) as vpool,
            tc.tile_pool(name="lstm", bufs=2) as lpool,
        ):
            # direct [p, cols] loads; eng picks the issuing DMA queue
            def ld(name, shape, d, eng=None):
                t = cpool.tile(shape, d, tag=name)
                (eng or nc.scalar).dma_start(t[:], D[name][:])
                return t

            # ---- conv weights first (sync queue, ahead of obs blocks) ----
            w1_s = ld('w1', [96, 64], dt.bfloat16, eng=nc.sync)
            cb1_s = ld('cb1', [128, 1], dt.float32, eng=nc.sync)
            w2_s = ld('w2', [128, 256], dt.bfloat16, eng=nc.sync)
            cb2_s = ld('cb2', [64, 1], dt.float32, eng=nc.sync)
            w3_s = ld('w3', [64, 576], dt.bfloat16, eng=nc.sync)
            cb3_s = ld('cb3', [128, 1], dt.float32, eng=nc.sync)

            zeros_s = cpool.tile([128, 512], dt.float32, tag="zeros")
            nc.vector.memset(zeros_s[:], 0.0)

            # ---- cross-phase activation tiles ----
            h1T_s = apool.tile([128, 8 * FPC], dt.bfloat16, tag="h1T")
            doutT_s = apool.tile([128, 4 * FPC], dt.bfloat16, tag="doutT")
            c3_s = apool.tile([128, FPC * 8], dt.bfloat16, tag="c3")

            # ============ conv trunk (one PSUM pool spans conv..lstm so
            # blocks 4-7 can interleave with LSTM stack A) ============
            psA_cm = tc.tile_pool(name="psall", bufs=2, space="PSUM")
            psA = psA_cm.__enter__()

            def emit_conv_block(blk, tb, pieces=None):
                """Emit one conv block. pieces=None -> inline (scalar relus);
                else append (mm_fn, relu_fn) closures with all-vector relus
                for interleaving into the LSTM step loop."""
                f0 = blk * FB
                tv = tb[:].rearrange("p (f by bx) -> p f by bx", by=16, bx=16)
                st = {}
                w2v = w2_s[:].rearrange("p (a b) -> p a b", a=4)
                w3v = w3_s[:].rearrange("p (a b) -> p a b", a=9)

                def c1_mm(a, b):
                    if 'rt' not in st:
                        st['rt'] = vpool.tile([128, FB * 90], dt.bfloat16, name="rtB",
                                              tag="rtile", bufs=1)
                    ncols = (b - a) * 90
                    ps1 = psA.tile([128, 512], dt.float32, name="ps1t", tag="ps1", bufs=3)
                    for j in range(4):
                        for dx in range(2):
                            mov = tv[:, a:b, j:j + 11:2, dx:dx + 15]
                            nc.tensor.matmul(
                                ps1[32 * j:32 * j + 32, :ncols],
                                w1_s[:, 32 * dx:32 * dx + 32],
                                mov, start=(dx == 0), stop=(dx == 1),
                                tile_position=(0, 32 * j))
                    return ps1

                def c1_relu(ps1, a, b, gi):
                    ncols = (b - a) * 90
                    dst = st['rt'][:, a * 90:b * 90]
                    if pieces is None and gi % 2 == 0:
                        nc.scalar.activation(dst, ps1[:, :ncols], AF.Relu,
                                             bias=cb1_s[:], scale=1.0)
                    else:
                        nc.vector.scalar_tensor_tensor(
                            dst, ps1[:, :ncols], cb1_s[:],
                            zeros_s[:, :ncols], ALU.add, ALU.max)

                def c2_mm(a, b):
                    if 'c2' not in st:
                        st['c2'] = vpool.tile([64, FB * 36], dt.bfloat16, name="c2B",
                                              tag="c2", bufs=1)
                    rv = st['rt'][:].rearrange("p (f m x) -> p f m x", m=6, x=15)
                    ncols = (b - a) * 36
                    ps2 = psA.tile([64, 512], dt.float32, name="ps2t", tag="ps2", bufs=2)
                    for kx in range(4):
                        mov = rv[:, a:b, :, kx:kx + 11:2]
                        nc.tensor.matmul(ps2[:, :ncols], w2v[:, kx, :], mov,
                                         start=(kx == 0), stop=(kx == 3))
                    return ps2

                def c2_relu(ps2, a, b):
                    ncols = (b - a) * 36
                    dst = st['c2'][:, a * 36:b * 36]
                    if pieces is None:
                        nc.scalar.activation(dst, ps2[:, :ncols], AF.Relu,
                                             bias=cb2_s[:], scale=1.0)
                    else:
                        nc.vector.scalar_tensor_tensor(
                            dst, ps2[:, :ncols], cb2_s[:],
                            zeros_s[0:64, :ncols], ALU.add, ALU.max)

                def c3_mm():
                    c2v = st['c2'][:].rearrange("p (f a b) -> p f a b", a=6, b=6)
                    ps3 = psA.tile([128, FB * 8], dt.float32, name="ps3t", tag="ps3", bufs=2)
                    for g in range(2):
                        for ki, (ky, kx) in enumerate(
                                [(y, x) for y in range(3) for x in range(3)]):
                            mov = c2v[:, :, 2 * g + ky:2 * g + ky + 2, kx:kx + 4]
                            nc.tensor.matmul(ps3[64 * g:64 * g + 64, :],
                                             w3v[:, 3 * ky + kx, :], mov,
                                             start=(ki == 0), stop=(ki == 8),
                                             tile_position=(0, 64 * g))
                    return ps3

                def c3_relu(ps3):
                    dst = c3_s[:, f0 * 8:(f0 + FB) * 8]
                    if pieces is None:
                        nc.scalar.activation(dst, ps3[:], AF.Relu,
                                             bias=cb3_s[:], scale=1.0)
                    else:
                        nc.vector.scalar_tensor_tensor(
                            dst, ps3[:], cb3_s[:],
                            zeros_s[:, 0:FB * 8], ALU.add, ALU.max)

                work = []
                for gi, (a, b) in enumerate(
                        [(i, min(i + 5, FB)) for i in range(0, FB, 5)]):
                    work.append(((lambda a=a, b=b: c1_mm(a, b)),
                                 (lambda ps, a=a, b=b, gi=gi: c1_relu(ps, a, b, gi))))
                for (a, b) in [(0, 12), (12, 24), (24, 32)]:
                    work.append(((lambda a=a, b=b: c2_mm(a, b)),
                                 (lambda ps, a=a, b=b: c2_relu(ps, a, b))))
                work.append((c3_mm, (lambda ps: c3_relu(ps))))
                if pieces is None:
                    for mm, relu in work:
                        relu(mm())
                else:
                    pieces.extend(work)

            for blk in range(4):
                tb = vpool.tile([96, FB * 256], dt.bfloat16, tag="tblk", bufs=4)
                if blk == 0:
                    # frame-chunked so the first conv group starts after ~250KB
                    for (a, b) in [(i, min(i + 5, FB)) for i in range(0, FB, 5)]:
                        nc.sync.dma_start(tb[:, a * 256:b * 256],
                                          D['obsT'][blk, :, a * 256:b * 256])
                else:
                    nc.sync.dma_start(tb[:], D['obsT'][blk])
                emit_conv_block(blk, tb)

            # ---- big weights queue on the sync ring BEHIND the obs blocks:
            # the blocked conv-tile buffers throttle them so obs streaming
            # keeps DMA priority; they land well before the MLP needs them ----
            dataT_s = ld('dataT', [128, 12 * FPC], dt.bfloat16, eng=nc.sync)
            lin1w_s = ld('lin1w', [128, 12 * 1024], dt.bfloat16, eng=nc.sync)
            b1c_s = ld('b1col', [128, 8], dt.float32)
            lin2w_s = ld('lin2w', [128, 8 * 512], dt.bfloat16, eng=nc.sync)
            b2c_s = ld('b2col', [128, 4], dt.float32)
            wih0_s = ld('wih0', [128, 24 * G4], dt.bfloat16, eng=nc.sync)
            b0_s = ld('b0row', [1, 2 * G4], dt.bfloat16)
            whh0_s = ld('whh0', [100, 2 * G4], dt.bfloat16, eng=nc.sync)
            wih1_s = ld('wih1', [100, 2 * G4], dt.bfloat16, eng=nc.sync)
            whh1_s = ld('whh1', [101, 2 * G4], dt.bfloat16, eng=nc.sync)
            fc1w_s = ld('fc1w', [100, 2 * 512], dt.bfloat16)
            fc1b_s = ld('fc1bcol', [128, 4], dt.float32)
            fc2w_s = ld('fc2w', [128, 4 * OUT], dt.bfloat16)
            fc2b_s = ld('fc2brow', [1, OUT], dt.bfloat16)
            i4_s = ld('i4rep', [128, 4], dt.bfloat16)
            id36_s = ld('ident36', [36, 36], dt.bfloat16)
            ones128_s = cpool.tile([1, 128], dt.bfloat16, tag="ones128")
            nc.gpsimd.memset(ones128_s[:], 1.0)
            ones4_s = cpool.tile([1, 4], dt.bfloat16, tag="ones4")
            nc.gpsimd.memset(ones4_s[:], 1.0)

            # ---- conv blocks 4-7: DMA now, compute interleaved with LSTM-A
            pieces = []
            for blk in range(4, NBLK):
                tb = vpool.tile([96, FB * 256], dt.bfloat16, tag="tblk", bufs=4)
                nc.sync.dma_start(tb[:], D['obsT'][blk])
                emit_conv_block(blk, tb)

            # =========================== MLP ===========================
            l1v = lin1w_s[:].rearrange("p (a g) -> p a g", a=12)
            dv = dataT_s[:].rearrange("p (c f) -> p c f", c=12)
            for m in range(8):
                ph = psA.tile([128, FPC], dt.float32, name="ph1", tag="ps1", bufs=3)
                for kc in range(12):
                    nc.tensor.matmul(ph[:], l1v[:, kc, 128 * m:128 * m + 128],
                                     dv[:, kc, :],
                                     start=(kc == 0), stop=(kc == 11))
                nc.scalar.activation(
                    h1T_s[:, FPC * m:FPC * (m + 1)],
                    ph[:], AF.Relu, bias=b1c_s[:, m:m + 1], scale=1.0)
            l2v = lin2w_s[:].rearrange("p (a g) -> p a g", a=8)
            for m in range(4):
                pd = psA.tile([128, FPC], dt.float32, name="pd1", tag="ps1", bufs=3)
                for kc in range(8):
                    nc.tensor.matmul(pd[:], l2v[:, kc, 128 * m:128 * m + 128],
                                     h1T_s[:, FPC * kc:FPC * (kc + 1)],
                                     start=(kc == 0), stop=(kc == 7))
                nc.vector.tensor_scalar_add(
                    doutT_s[:, FPC * m:FPC * (m + 1)],
                    pd[:], b2c_s[:, m:m + 1])

            # ===================== XW precompute =====================
            xwpad = []
            for l in range(2):
                xwp = apool.tile([128, 8 * G4], dt.bfloat16, tag=f"xwpad{l}")
                xwpad.append(xwp)
            wih0v = wih0_s[:].rearrange("p (a c g) -> p a c g", a=2, c=12)
            b0v = b0_s[:].rearrange("p (a g) -> p a g", a=2)

            def emit_xw(lst):
                goff = lst * 128
                psx = psA.tile([128, G4], dt.float32, name="psx", tag="ps1", bufs=3)
                for kc in range(8):
                    stat = c3_s[:, goff * 8 + kc: goff * 8 + kc + 8 * 127 + 1: 8]
                    nc.tensor.matmul(psx[:], stat, wih0v[:, lst, kc, :],
                                     start=(kc == 0), stop=False)
                for kc in range(8, 12):
                    stat = doutT_s[:, FPC * (kc - 8) + goff:
                                   FPC * (kc - 8) + goff + 128]
                    nc.tensor.matmul(psx[:], stat, wih0v[:, lst, kc, :],
                                     start=False, stop=False)
                nc.tensor.matmul(psx[:], ones128_s[:], b0v[:, lst, :],
                                 start=False, stop=True)
                xwc = lpool.tile([128, G4], dt.bfloat16, tag="xwc")
                nc.scalar.activation(xwc[:], psx[:], AF.Copy, bias=0.0, scale=1.0)
                # scatter via DRAM roundtrip (multi-level-partition SBUF
                # DMAs read garbage): rows (b,t)=32b+4tg+tau -> pad row
                # 32tau+b, col 400tg+g
                nc.sync.dma_start(xw_scr[lst], xwc[:])
                srcv = xw_scr[lst].rearrange("(b tg tau) g -> tau b tg g",
                                             b=4, tg=8)
                padv = xwpad[lst][:].rearrange("(tau q) (tg g) -> tau q tg g",
                                               tau=4, tg=8)
                for tau in range(4):
                    nc.sync.dma_start(padv[tau, 0:4], srcv[tau])

            emit_xw(0)
            emit_xw(1)
            psA_cm.__exit__(None, None, None)
            psB_cm = tc.tile_pool(name="pslstm", bufs=2, space="PSUM")
            psB = psB_cm.__enter__()

            # ========================= LSTM =========================
            hT = []
            for i in range(2):
                hTt = lpool.tile([101, 36], dt.bfloat16, tag=f"hT{i}", bufs=1)
                nc.scalar.dma_start(hTt[:], D['hT_init'][:])
                hT.append(hTt)
            sC = lpool.tile([36, 100], dt.float32, tag="sC", bufs=1)
            nc.gpsimd.memset(sC[:], 0.0)
            # persistent o-gate / cell-tanh tiles (zeros so inactive skew rows
            # transpose to h=0 on the first steps)
            To_t = lpool.tile([36, 100], dt.bfloat16, tag="Tot", bufs=1)
            nc.gpsimd.memset(To_t[:], 0.0)
            th_t = lpool.tile([36, 100], dt.bfloat16, tag="tht16", bufs=1)
            nc.gpsimd.memset(th_t[:], 0.0)
            ToT_s = lpool.tile([100, 36], dt.bfloat16, tag="ToT", bufs=1)
            qsave = lpool.tile([100, 4], dt.bfloat16, tag="qsave", bufs=1)

            whh0v = whh0_s[:].rearrange("p (a g) -> p a g", a=2)
            wih1v = wih1_s[:].rearrange("p (a g) -> p a g", a=2)
            whh1v = whh1_s[:].rearrange("p (a g) -> p a g", a=2)

            if True:
                psl = psB
                pst_o = psl.tile([100, 36], dt.bfloat16, tag="psto", bufs=1)
                pst_th = psl.tile([100, 36], dt.bfloat16, tag="pstth", bufs=1)
                Gs = {}

                def alloc_G(s):
                    """Allocate step-s gate PSUM and pre-accumulate its xw
                    contribution; runs on the PE while the previous step's
                    cell phase occupies Vector/Scalar."""
                    G = psl.tile([36, G4], dt.float32, name="G", tag="gpsum", bufs=4)
                    Gs[s] = G
                    if s <= 63:
                        lo = 0 if s < 32 else 1
                        t0m = s % 32
                        xs = xwpad[lo][:].rearrange("p (tg g) -> p tg g", tg=8)
                        q4 = 32 * (t0m % 4)
                        nc.tensor.matmul(G[0:4, :], i4_s[q4:q4 + 4, :],
                                         xs[q4:q4 + 4, t0m // 4, :],
                                         start=True, stop=False,
                                         tile_position=(q4, 0))

                alloc_G(0)
                for s_ in range(NSTEP):
                    l0_act = s_ <= 63
                    l1_act = 1 <= s_
                    lo = 0 if s_ < 32 else 1
                    l1i = 0 if (s_ - 1) < 32 else 1
                    hp = hT[(s_ - 1) % 2]
                    hn = hT[s_ % 2]
                    p0, p1 = (0, 4) if s_ == 0 else ((32, 36) if s_ == 64 else (0, 36))
                    G = Gs[s_]

                    # h-recurrence matmul leads; l1 follows
                    if l0_act:
                        nc.tensor.matmul(G[0:4, :], hp[0:100, 0:4], whh0v[:, lo, :],
                                         start=False, stop=True, tile_position=(0, 0))
                    if l1_act:
                        nc.tensor.matmul(G[32:36, :], hp[0:100, 0:4], wih1v[:, l1i, :],
                                         start=True, stop=False, tile_position=(0, 32))
                        nc.tensor.matmul(G[32:36, :], hp[0:101, 32:36], whh1v[:, l1i, :],
                                         start=False, stop=True, tile_position=(0, 32))

                    # conv-B matmul pieces slot into the cell-phase PE idle
                    npc = 2 if s_ < 14 else 1
                    relus_pending = []
                    if s_ < 32:
                        for _ in range(npc):
                            if pieces:
                                mm, relu = pieces.pop(0)
                                ps = mm()
                                relus_pending.append((relu, ps))

                    T = lpool.tile([36, G4], dt.float32, tag="tanhT")
                    # (f,i,g) on the critical chain; o off-chain into bf16,
                    # transposed early so h is produced directly as hT
                    nc.scalar.activation(T[p0:p1, 0:300], G[p0:p1, 0:300], AF.Tanh,
                                         bias=0.0, scale=0.5)
                    nc.scalar.activation(To_t[p0:p1, :], G[p0:p1, 300:400], AF.Tanh,
                                         bias=0.0, scale=0.5)
                    nc.tensor.transpose(pst_o[:], To_t[:], id36_s[:])
                    u = lpool.tile([36, 100], dt.float32, tag="ut")
                    v = lpool.tile([36, 100], dt.float32, tag="vt")
                    cn = lpool.tile([36, 100], dt.float32, tag="cnt")
                    nc.vector.scalar_tensor_tensor(u[p0:p1, :], T[p0:p1, 0:100], 1.0,
                                                   sC[p0:p1, :], ALU.add, ALU.mult)
                    nc.vector.scalar_tensor_tensor(v[p0:p1, :], T[p0:p1, 100:200], 1.0,
                                                   T[p0:p1, 200:300], ALU.add, ALU.mult)
                    nc.vector.scalar_tensor_tensor(cn[p0:p1, :], v[p0:p1, :], 0.5,
                                                   u[p0:p1, :], ALU.mult, ALU.add)
                    nc.scalar.activation(th_t[p0:p1, :], cn[p0:p1, :], AF.Tanh,
                                         bias=0.0, scale=1.0)
                    nc.vector.tensor_scalar_mul(sC[p0:p1, :], cn[p0:p1, :], 0.5)
                    nc.vector.tensor_copy(ToT_s[:], pst_o[:])
                    for relu, ps in relus_pending:
                        relu(ps)
                    nc.tensor.transpose(pst_th[:], th_t[:], id36_s[:])
                    if s_ + 1 < NSTEP:
                        alloc_G(s_ + 1)
                    # hT = (To^T + 1) * th^T  — straight into the h history
                    nc.vector.scalar_tensor_tensor(hn[0:100, :], ToT_s[:], 1.0,
                                                   pst_th[:], ALU.add, ALU.mult)
                    if s_ == 32:
                        nc.scalar.copy(qsave[:], hn[0:100, 32:36])

            psB_cm.__exit__(None, None, None)

            # ========================= head =========================
            with tc.tile_pool(name="pshead", bufs=1, space="PSUM") as psh:
                hlast = hT[(NSTEP - 1) % 2]
                psf1 = psh.tile([128, 16], dt.float32, tag="psf1")
                fc1v = fc1w_s[:].rearrange("p (a g) -> p a g", a=2)
                for m in range(4):
                    nc.tensor.matmul(psf1[:, 4 * m:4 * m + 4],
                                     fc1v[:, 0, 128 * m:128 * m + 128],
                                     hlast[0:100, 32:36], start=True, stop=False)
                    nc.tensor.matmul(psf1[:, 4 * m:4 * m + 4],
                                     fc1v[:, 1, 128 * m:128 * m + 128],
                                     qsave[:], start=False, stop=True)
                z1 = lpool.tile([128, 16], dt.bfloat16, tag="z1t", bufs=1)
                for m in range(4):
                    # relu on vector (avoids an ACT table swap away from Tanh)
                    nc.vector.scalar_tensor_tensor(
                        z1[:, 4 * m:4 * m + 4], psf1[:, 4 * m:4 * m + 4],
                        fc1b_s[:, m:m + 1], zeros_s[:, 0:4], ALU.add, ALU.max)
                psf2 = psh.tile([4, OUT], dt.float32, tag="psf2")
                fc2v = fc2w_s[:].rearrange("p (a g) -> p a g", a=4)
                for m in range(4):
                    nc.tensor.matmul(psf2[:], z1[:, 4 * m:4 * m + 4], fc2v[:, m, :],
                                     start=(m == 0), stop=False)
                nc.tensor.matmul(psf2[:], ones4_s[:], fc2b_s[:],
                                 start=False, stop=True)
                ot = lpool.tile([4, OUT], dt.float32, tag="outt", bufs=1)
                nc.scalar.copy(ot[:], psf2[:])
                nc.sync.dma_start(out_d[:], ot[:])

    _fix_sync_waits(nc)
    return nc


_NC_CACHE = {}


def _run(inputs):
    inputs = {k: np.asarray(v) for k, v in inputs.items()}
    winp = {k: (np.asarray(v, np.float32) if np.asarray(v).ndim else v)
            for k, v in inputs.items()}
    w = _prep_weights(winp)
    key = 'nc'
    if key not in _NC_CACHE:
        _NC_CACHE[key] = _build_nc()
    nc = _NC_CACHE[key]
    in_maps = [_prep_core_inputs(inputs, w, k) for k in range(N_CORES)]
    res = run_bass_kernel_spmd(nc, in_maps, core_ids=list(range(N_CORES)))
    return res


def kernel(**inputs):
    res = _run(inputs)
    out = np.concatenate([res.results[k]['out'] for k in range(N_CORES)], 0)
    return out.astype(np.float32)


# revision 41
# speedup vs baseline: 1.0068x; 1.0068x over previous
"""Trainium2 Bass kernel for nn_NatureCNN (CNN trunk + MLP + 2x2-layer LSTM + FC head).

Sharding: data-parallel over the segment axis. Core k owns segments
b in [4k, 4k+4) of both LSTM stacks, i.e. frames [128k,128k+128) (rnns
block A) and [1024+128k, +128) (rnn block B) -- 256 frames/core, fully
independent per core (zero communication). Host stacks the per-core
[4, 130] outputs.

On-chip layouts (channels-on-partitions):
  obsT   [48=(ic,ry,rx), 256f, (by,bx)=256]  phase-decomposed 4x4 blocks, bf16
  conv1  K=48, PSUM-accumulate over (dy,dx) block shifts; 4 col-tiles by
         output-row group j (rows {j, j+2, .., j+10}) -> R [(j,oc1)=128, ...]
  conv2  K=128=(ky2,ic2) directly on R; accumulate kx2 -> C2 [64, (f,6,6)]
  conv3  K=64; accumulate (ky3,kx3); 2 col-tiles by py3-pair -> C3
  MLP    dataT [1536, f] bf16 lin1; fp32r lin2 -> doutT
  LSTM   layer-skewed steps; gate order (f,i,g,o); gates layer0 at PSUM
         rows 0:4, layer1 at 32:36; xw contribution pre-accumulated into
         next step's PSUM via i4 extract while the cell phase runs;
         tanh split (f,i,g) on-chain / (o) off-chain;
         sigmoid(x)=(tanh(x/2)+1)/2; c stored as c/2 ("sC"), h stored
         as 2h (0.5 folded into weights); PE-transpose of h each step.
  head   feature-major fc1/fc2, batch on the moving free dim.

All weights are pre-laid out host-side in their exact SBUF layout so
every DMA is a contiguous [p, cols] copy (one large descriptor per
partition). Weight loads are issued on the scalar-engine DMA queue so
they don't head-of-line block the obs-block stream on the sync queue.
"""

import numpy as np
import ml_dtypes

import concourse.bass as bass
import concourse.mybir as mybir
import concourse.tile as tile
from concourse.bass_utils import run_bass_kernel_spmd

dt = mybir.dt
AF = mybir.ActivationFunctionType
ALU = mybir.AluOpType

N_CORES = 8
S, H = 32, 100
B_LOC = 4                          # segments per core
FPC = 2 * B_LOC * S                # 256 frames per core
FB = 32                            # frames per conv block
NBLK = FPC // FB
G4 = 4 * H                         # 400
DATA_DIM, RIN, OUT = 1500, 1536, 130
NSTEP = 2 * S + 1                  # 65 layer-skewed steps

bf = ml_dtypes.bfloat16

MAX_WAITS = 1
_ctr = [0]


def _fix_sync_waits(nc):
    """This env's walrus rejects >1 sem-wait per instruction; hoist extras
    onto same-engine NoOps inserted immediately before, preserving order."""
    for f in nc.m.functions:
        for bb in f.blocks:
            new_list = []
            for ins in bb.instructions:
                si = ins.sync_info
                if si is not None and si.on_wait and len(si.on_wait) > MAX_WAITS:
                    waits = list(si.on_wait)
                    for i in range(0, len(waits) - MAX_WAITS, MAX_WAITS):
                        _ctr[0] += 1
                        nop = mybir.InstNoOp(name=f"waitfix-{_ctr[0]}", ins=[], outs=[])
                        nop.engine = ins.engine
                        nop.sync_info = mybir.SyncInfo(
                            on_wait=waits[i : i + MAX_WAITS], on_update=[])
                        new_list.append(nop)
                        nc.register_instruction(nop, overwrite=True)
                    si.on_wait = waits[len(waits) - MAX_WAITS :]
                    ins.sync_info = si
                new_list.append(ins)
            bb.instructions = new_list


# --------------------------------------------------------------------------
# host-side weight/layout prep
# --------------------------------------------------------------------------

def _gate_perm():
    # torch gate order (i, f, g, o) -> ours (f, i, g, o)
    return np.concatenate([np.arange(100, 200), np.arange(0, 100),
                           np.arange(200, 300), np.arange(300, 400)])


def _feat_ch_order():
    """our featT row R -> original feat channel index."""
    orig = np.zeros(RIN, np.int64)
    for R in range(1024):
        c, rem = divmod(R, 128)
        g, oc3 = divmod(rem, 64)
        orig[R] = oc3 * 16 + (2 * g + c // 4) * 4 + (c % 4)
    orig[1024:] = np.arange(1024, RIN)
    return orig


def _prep_weights(inp):
    w = {}
    gp = _gate_perm()
    ch = _feat_ch_order()

    c1 = inp['conv1_w'].reshape(32, 3, 2, 4, 2, 4)   # oc, ic, dy, ry, dx, rx
    w1 = c1.transpose(2, 4, 1, 3, 5, 0).reshape(2, 2, 48, 32)  # dy, dx, p, oc
    # K=96 stationary: rows 0:48 dy=0 taps, rows 48:96 dy=1 taps; col block
    # per dx. Pairs with the row-shifted duplicate half of obsT.
    w1d = np.concatenate([w1[0], w1[1]], axis=1)     # dx, 96, 32
    w['w1'] = np.ascontiguousarray(w1d.transpose(1, 0, 2).reshape(96, 64)).astype(bf)
    w['cb1'] = np.tile(inp['conv1_b'], 4).reshape(128, 1).astype(np.float32)
    w2 = inp['conv2_w'].transpose(3, 2, 1, 0).reshape(4, 128, 64)
    w['w2'] = np.ascontiguousarray(w2.transpose(1, 0, 2).reshape(128, 256)).astype(bf)
    w['cb2'] = inp['conv2_b'].reshape(64, 1).astype(np.float32)
    w3 = inp['conv3_w'].transpose(2, 3, 1, 0).reshape(9, 64, 64)
    w['w3'] = np.ascontiguousarray(w3.transpose(1, 0, 2).reshape(64, 576)).astype(bf)
    w['cb3'] = np.tile(inp['conv3_b'], 2).reshape(128, 1).astype(np.float32)

    l1 = np.zeros((RIN, 1024), np.float32)
    l1[:DATA_DIM] = inp['lin1_w'].T
    w['lin1w'] = np.ascontiguousarray(
        l1.reshape(12, 128, 1024).transpose(1, 0, 2).reshape(128, 12 * 1024)).astype(bf)
    w['b1col'] = inp['lin1_b'].reshape(8, 128).T.copy().astype(np.float32)
    w['lin2w'] = np.ascontiguousarray(
        inp['lin2_w'].T.reshape(8, 128, 512).transpose(1, 0, 2)
        .reshape(128, 8 * 512)).astype(bf)
    w['b2col'] = inp['lin2_b'].reshape(4, 128).T.copy().astype(np.float32)

    def lstm_prep(pfx):
        # gate order (f,i,g,o); g rows (200:300) pre-scaled by 2 so a single
        # tanh(x/2) computes sigmoid-halves for f,i,o and tanh for g.
        wih0 = inp[f'{pfx}_wih0'][gp].copy()
        wih0[200:300] *= 2.0
        wih0T = wih0.T[ch].copy()                     # [1536, 400]
        b0 = (inp[f'{pfx}_bih0'] + inp[f'{pfx}_bhh0'])[gp].copy()
        b0[200:300] *= 2.0
        whh0 = inp[f'{pfx}_whh0'][gp].copy()
        whh0[200:300] *= 2.0
        wih1 = inp[f'{pfx}_wih1'][gp].copy()
        wih1[200:300] *= 2.0
        whh1 = inp[f'{pfx}_whh1'][gp].copy()
        whh1[200:300] *= 2.0
        b1 = (inp[f'{pfx}_bih1'] + inp[f'{pfx}_bhh1'])[gp].copy()
        b1[200:300] *= 2.0
        whh1ext = np.concatenate([whh1.T * 0.5, b1[None, :]], 0)
        return wih0T, b0, whh0.T * 0.5, wih1.T * 0.5, whh1ext

    a = [lstm_prep('rnns'), lstm_prep('rnn')]
    # SBUF layouts directly:
    wih0 = np.stack([x[0] for x in a]).reshape(2, 12, 128, G4)   # lst, kc, p, g
    w['wih0'] = np.ascontiguousarray(
        wih0.transpose(2, 0, 1, 3).reshape(128, 2 * 12 * G4)).astype(bf)
    w['b0row'] = np.stack([x[1] for x in a]).reshape(1, 2 * G4).astype(bf)
    w['whh0'] = np.ascontiguousarray(
        np.stack([x[2] for x in a]).transpose(1, 0, 2).reshape(100, 2 * G4)).astype(bf)
    w['wih1'] = np.ascontiguousarray(
        np.stack([x[3] for x in a]).transpose(1, 0, 2).reshape(100, 2 * G4)).astype(bf)
    w['whh1'] = np.ascontiguousarray(
        np.stack([x[4] for x in a]).transpose(1, 0, 2).reshape(101, 2 * G4)).astype(bf)

    fc1 = np.stack([inp['fc1_w'][:, :100].T * 0.5,
                    inp['fc1_w'][:, 100:].T * 0.5])              # [2, 100, 512]
    w['fc1w'] = np.ascontiguousarray(
        fc1.transpose(1, 0, 2).reshape(100, 1024)).astype(bf)
    w['fc1bcol'] = inp['fc1_b'].reshape(4, 128).T.copy().astype(np.float32)
    w['fc2w'] = np.ascontiguousarray(
        inp['fc2_w'].T.reshape(4, 128, OUT).transpose(1, 0, 2)
        .reshape(128, 4 * OUT)).astype(bf)
    w['fc2brow'] = inp['fc2_b'].reshape(1, OUT).astype(bf)

    i4 = np.zeros((128, 4), np.float32)
    for k in range(4):
        i4[32 * k : 32 * k + 4] = np.eye(4)
    w['i4rep'] = i4.astype(bf)
    w['ident36'] = np.eye(36).astype(bf)
    w['hT_init'] = np.concatenate(
        [np.zeros((100, 36), np.float32), np.ones((1, 36), np.float32)]).astype(bf)
    return w


def _prep_core_inputs(inp, w, k):
    idx = np.concatenate([np.arange(128 * k, 128 * k + 128),
                          np.arange(1024 + 128 * k, 1024 + 128 * k + 128)])
    obs = np.asarray(inp['observations'], np.float32)[idx]
    obsT = (obs.reshape(FPC, 3, 16, 4, 16, 4)
            .transpose(1, 3, 5, 0, 2, 4)
            .reshape(48, FPC, 16, 16)).astype(bf)
    # duplicate with block-rows shifted by one (the dy=1 phase) so conv1 can
    # contract K=96 in a single accumulation pair; block-major for 16KB descs
    sh = np.zeros_like(obsT)
    sh[:, :, :15, :] = obsT[:, :, 1:, :]
    obsT = np.concatenate([obsT, sh], axis=0).reshape(96, NBLK, FB * 256)
    obsT = np.ascontiguousarray(obsT.transpose(1, 0, 2))
    data = np.asarray(inp['data'], np.float32)[idx]
    dT = np.zeros((RIN, FPC), np.float32)
    dT[:DATA_DIM] = data.T
    dsb = np.ascontiguousarray(
        dT.reshape(12, 128, FPC).transpose(1, 0, 2).reshape(128, 12 * FPC)).astype(bf)
    m = {'obsT': obsT, 'dataT': dsb}
    m.update(w)
    return m


# --------------------------------------------------------------------------
# kernel IR
# --------------------------------------------------------------------------

def _build_nc():
    nc = bass.Bass("TRN2", target_bir_lowering=False, debug=False,
                   num_devices=N_CORES)

    D = {}
    def inp(name, shape, d):
        D[name] = nc.dram_tensor(name, shape, d, kind="ExternalInput")

    inp('obsT', [NBLK, 96, FB * 256], dt.bfloat16)
    inp('dataT', [128, 12 * FPC], dt.bfloat16)
    inp('w1', [96, 64], dt.bfloat16)
    inp('cb1', [128, 1], dt.float32)
    inp('w2', [128, 256], dt.bfloat16)
    inp('cb2', [64, 1], dt.float32)
    inp('w3', [64, 576], dt.bfloat16)
    inp('cb3', [128, 1], dt.float32)
    inp('lin1w', [128, 12 * 1024], dt.bfloat16)
    inp('b1col', [128, 8], dt.float32)
    inp('lin2w', [128, 8 * 512], dt.bfloat16)
    inp('b2col', [128, 4], dt.float32)
    inp('wih0', [128, 24 * G4], dt.bfloat16)
    inp('b0row', [1, 2 * G4], dt.bfloat16)
    inp('whh0', [100, 2 * G4], dt.bfloat16)
    inp('wih1', [100, 2 * G4], dt.bfloat16)
    inp('whh1', [101, 2 * G4], dt.bfloat16)
    inp('fc1w', [100, 2 * 512], dt.bfloat16)
    inp('fc1bcol', [128, 4], dt.float32)
    inp('fc2w', [128, 4 * OUT], dt.bfloat16)
    inp('fc2brow', [1, OUT], dt.bfloat16)
    inp('i4rep', [128, 4], dt.bfloat16)
    inp('ident36', [36, 36], dt.bfloat16)
    inp('hT_init', [101, 36], dt.bfloat16)

    out_d = nc.dram_tensor('out', [B_LOC, OUT], dt.float32, kind="ExternalOutput")
    xw_scr = nc.dram_tensor('xw_scr', [2, 128, G4], dt.bfloat16)  # internal scratch

    with tile.TileContext(nc) as tc:
        with (
            tc.tile_pool(name="const", bufs=1) as cpool,
            tc.tile_pool(name="acts", bufs=1) as apool,
            tc.tile_pool(name="conv", bufs=2) as vpool,
            tc.tile_pool(name="lstm", bufs=2) as lpool,
        ):
            # direct [p, cols] loads; eng picks the issuing DMA queue
            def ld(name, shape, d, eng=None):
                t = cpool.tile(shape, d, tag=name)
                (eng or nc.scalar).dma_start(t[:], D[name][:])
                return t

            # ---- conv weights first (sync queue, ahead of obs blocks) ----
            w1_s = ld('w1', [96, 64], dt.bfloat16, eng=nc.sync)
            cb1_s = ld('cb1', [128, 1], dt.float32, eng=nc.sync)
            w2_s = ld('w2', [128, 256], dt.bfloat16, eng=nc.sync)
            cb2_s = ld('cb2', [64, 1], dt.float32, eng=nc.sync)
            w3_s = ld('w3', [64, 576], dt.bfloat16, eng=nc.sync)
            cb3_s = ld('cb3', [128, 1], dt.float32, eng=nc.sync)

            zeros_s = cpool.tile([128, 512], dt.float32, tag="zeros")
            nc.vector.memset(zeros_s[:], 0.0)

            # ---- cross-phase activation tiles ----
            h1T_s = apool.tile([128, 8 * FPC], dt.bfloat16, tag="h1T")
            doutT_s = apool.tile([128, 4 * FPC], dt.bfloat16, tag="doutT")
            c3_s = apool.tile([128, FPC * 8], dt.bfloat16, tag="c3")

            # ============ conv trunk (one PSUM pool spans conv..lstm so
            # blocks 4-7 can interleave with LSTM stack A) ============
            psA_cm = tc.tile_pool(name="psall", bufs=2, space="PSUM")
            psA = psA_cm.__enter__()

            def emit_conv_block(blk, tb, pieces=None):
                """Emit one conv block. pieces=None -> inline (scalar relus);
                else append (mm_fn, relu_fn) closures with all-vector relus
                for interleaving into the LSTM step loop."""
                f0 = blk * FB
                tv = tb[:].rearrange("p (f by bx) -> p f by bx", by=16, bx=16)
                st = {}
                w2v = w2_s[:].rearrange("p (a b) -> p a b", a=4)
                w3v = w3_s[:].rearrange("p (a b) -> p a b", a=9)

                def c1_mm(a, b):
                    if 'rt' not in st:
                        st['rt'] = vpool.tile([128, FB * 90], dt.bfloat16, name="rtB",
                                              tag="rtile", bufs=1)
                    ncols = (b - a) * 90
                    ps1 = psA.tile([128, 512], dt.float32, name="ps1t", tag="ps1", bufs=3)
                    for j in range(4):
                        for dx in range(2):
                            mov = tv[:, a:b, j:j + 11:2, dx:dx + 15]
                            nc.tensor.matmul(
                                ps1[32 * j:32 * j + 32, :ncols],
                                w1_s[:, 32 * dx:32 * dx + 32],
                                mov, start=(dx == 0), stop=(dx == 1),
                                tile_position=(0, 32 * j))
                    return ps1

                def c1_relu(ps1, a, b, gi):
                    ncols = (b - a) * 90
                    dst = st['rt'][:, a * 90:b * 90]
                    if pieces is None and gi % 2 == 0:
                        nc.scalar.activation(dst, ps1[:, :ncols], AF.Relu,
                                             bias=cb1_s[:], scale=1.0)
                    else:
                        nc.vector.scalar_tensor_tensor(
                            dst, ps1[:, :ncols], cb1_s[:],
                            zeros_s[:, :ncols], ALU.add, ALU.max)

                def c2_mm(a, b):
                    if 'c2' not in st:
                        st['c2'] = vpool.tile([64, FB * 36], dt.bfloat16, name="c2B",
                                              tag="c2", bufs=1)
                    rv = st['rt'][:].rearrange("p (f m x) -> p f m x", m=6, x=15)
                    ncols = (b - a) * 36
                    ps2 = psA.tile([64, 512], dt.float32, name="ps2t", tag="ps2", bufs=2)
                    for kx in range(4):
                        mov = rv[:, a:b, :, kx:kx + 11:2]
                        nc.tensor.matmul(ps2[:, :ncols], w2v[:, kx, :], mov,
                                         start=(kx == 0), stop=(kx == 3))
                    return ps2

                def c2_relu(ps2, a, b):
                    ncols = (b - a) * 36
                    dst = st['c2'][:, a * 36:b * 36]
                    if pieces is None:
                        nc.scalar.activation(dst, ps2[:, :ncols], AF.Relu,
                                             bias=cb2_s[:], scale=1.0)
                    else:
                        nc.vector.scalar_tensor_tensor(
                            dst, ps2[:, :ncols], cb2_s[:],
                            zeros_s[0:64, :ncols], ALU.add, ALU.max)

                def c3_mm():
                    c2v = st['c2'][:].rearrange("p (f a b) -> p f a b", a=6, b=6)
                    ps3 = psA.tile([128, FB * 8], dt.float32, name="ps3t", tag="ps3", bufs=2)
                    for g in range(2):
                        for ki, (ky, kx) in enumerate(
                                [(y, x) for y in range(3) for x in range(3)]):
                            mov = c2v[:, :, 2 * g + ky:2 * g + ky + 2, kx:kx + 4]
                            nc.tensor.matmul(ps3[64 * g:64 * g + 64, :],
                                             w3v[:, 3 * ky + kx, :], mov,
                                             start=(ki == 0), stop=(ki == 8),
                                             tile_position=(0, 64 * g))
                    return ps3

                def c3_relu(ps3):
                    dst = c3_s[:, f0 * 8:(f0 + FB) * 8]
                    if pieces is None:
                        nc.scalar.activation(dst, ps3[:], AF.Relu,
                                             bias=cb3_s[:], scale=1.0)
                    else:
                        nc.vector.scalar_tensor_tensor(
                            dst, ps3[:], cb3_s[:],
                            zeros_s[:, 0:FB * 8], ALU.add, ALU.max)

                work = []
                for gi, (a, b) in enumerate(
                        [(i, min(i + 5, FB)) for i in range(0, FB, 5)]):
                    work.append(((lambda a=a, b=b: c1_mm(a, b)),
                                 (lambda ps, a=a, b=b, gi=gi: c1_relu(ps, a, b, gi))))
                for (a, b) in [(0, 12), (12, 24), (24, 32)]:
                    work.append(((lambda a=a, b=b: c2_mm(a, b)),
                                 (lambda ps, a=a, b=b: c2_relu(ps, a, b))))
                work.append((c3_mm, (lambda ps: c3_relu(ps))))
                if pieces is None:
                    for mm, relu in work:
                        relu(mm())
                else:
                    pieces.extend(work)

            for blk in range(4):
                tb = vpool.tile([96, FB * 256], dt.bfloat16, tag="tblk", bufs=4)
                if blk == 0:
                    # frame-chunked so the first conv group starts after ~250KB
                    for (a, b) in [(i, min(i + 5, FB)) for i in range(0, FB, 5)]:
                        nc.sync.dma_start(tb[:, a * 256:b * 256],
                                          D['obsT'][blk, :, a * 256:b * 256])
                else:
                    nc.sync.dma_start(tb[:], D['obsT'][blk])
                emit_conv_block(blk, tb)

            # ---- big weights queue on the sync ring BEHIND the obs blocks:
            # the blocked conv-tile buffers throttle them so obs streaming
            # keeps DMA priority; they land well before the MLP needs them ----
            dataT_s = ld('dataT', [128, 12 * FPC], dt.bfloat16, eng=nc.sync)
            lin1w_s = ld('lin1w', [128, 12 * 1024], dt.bfloat16, eng=nc.sync)
            b1c_s = ld('b1col', [128, 8], dt.float32)
            lin2w_s = ld('lin2w', [128, 8 * 512], dt.bfloat16, eng=nc.sync)
            b2c_s = ld('b2col', [128, 4], dt.float32)
            wih0_s = ld('wih0', [128, 24 * G4], dt.bfloat16, eng=nc.sync)
            b0_s = ld('b0row', [1, 2 * G4], dt.bfloat16)
            whh0_s = ld('whh0', [100, 2 * G4], dt.bfloat16, eng=nc.sync)
            wih1_s = ld('wih1', [100, 2 * G4], dt.bfloat16, eng=nc.sync)
            whh1_s = ld('whh1', [101, 2 * G4], dt.bfloat16, eng=nc.sync)
            fc1w_s = ld('fc1w', [100, 2 * 512], dt.bfloat16)
            fc1b_s = ld('fc1bcol', [128, 4], dt.float32)
            fc2w_s = ld('fc2w', [128, 4 * OUT], dt.bfloat16)
            fc2b_s = ld('fc2brow', [1, OUT], dt.bfloat16)
            i4_s = ld('i4rep', [128, 4], dt.bfloat16)
            id36_s = ld('ident36', [36, 36], dt.bfloat16)
            ones128_s = cpool.tile([1, 128], dt.bfloat16, tag="ones128")
            nc.gpsimd.memset(ones128_s[:], 1.0)
            ones4_s = cpool.tile([1, 4], dt.bfloat16, tag="ones4")
            nc.gpsimd.memset(ones4_s[:], 1.0)

            # ---- conv blocks 4-7: DMA now, compute interleaved with LSTM-A
            pieces = []
            for blk in range(4, NBLK):
                tb = vpool.tile([96, FB * 256], dt.bfloat16, tag="tblk", bufs=4)
                nc.sync.dma_start(tb[:], D['obsT'][blk])
                emit_conv_block(blk, tb)

            # =========================== MLP ===========================
            l1v = lin1w_s[:].rearrange("p (a g) -> p a g", a=12)
            dv = dataT_s[:].rearrange("p (c f) -> p c f", c=12)
            for m in range(8):
                ph = psA.tile([128, FPC], dt.float32, name="ph1", tag="ps1", bufs=3)
                for kc in range(12):
                    nc.tensor.matmul(ph[:], l1v[:, kc, 128 * m:128 * m + 128],
                                     dv[:, kc, :],
                                     start=(kc == 0), stop=(kc == 11))
                nc.scalar.activation(
                    h1T_s[:, FPC * m:FPC * (m + 1)],
                    ph[:], AF.Relu, bias=b1c_s[:, m:m + 1], scale=1.0)
            l2v = lin2w_s[:].rearrange("p (a g) -> p a g", a=8)
            for m in range(4):
                pd = psA.tile([128, FPC], dt.float32, name="pd1", tag="ps1", bufs=3)
                for kc in range(8):
                    nc.tensor.matmul(pd[:], l2v[:, kc, 128 * m:128 * m + 128],
                                     h1T_s[:, FPC * kc:FPC * (kc + 1)],
                                     start=(kc == 0), stop=(kc == 7))
                nc.vector.tensor_scalar_add(
                    doutT_s[:, FPC * m:FPC * (m + 1)],
                    pd[:], b2c_s[:, m:m + 1])

            # ===================== XW precompute =====================
            xwpad = []
            for l in range(2):
                xwp = apool.tile([128, 8 * G4], dt.bfloat16, tag=f"xwpad{l}")
                xwpad.append(xwp)
            wih0v = wih0_s[:].rearrange("p (a c g) -> p a c g", a=2, c=12)
            b0v = b0_s[:].rearrange("p (a g) -> p a g", a=2)

            def emit_xw(lst):
                goff = lst * 128
                psx = psA.tile([128, G4], dt.float32, name="psx", tag="ps1", bufs=3)
                for kc in range(8):
                    stat = c3_s[:, goff * 8 + kc: goff * 8 + kc + 8 * 127 + 1: 8]
                    nc.tensor.matmul(psx[:], stat, wih0v[:, lst, kc, :],
                                     start=(kc == 0), stop=False)
                for kc in range(8, 12):
                    stat = doutT_s[:, FPC * (kc - 8) + goff:
                                   FPC * (kc - 8) + goff + 128]
                    nc.tensor.matmul(psx[:], stat, wih0v[:, lst, kc, :],
                                     start=False, stop=False)
                nc.tensor.matmul(psx[:], ones128_s[:], b0v[:, lst, :],
                                 start=False, stop=True)
                xwc = lpool.tile([128, G4], dt.bfloat16, tag="xwc")
                nc.scalar.activation(xwc[:], psx[:], AF.Copy, bias=0.0, scale=1.0)
                # scatter via DRAM roundtrip (multi-level-partition SBUF
                # DMAs read garbage): rows (b,t)=32b+4tg+tau -> pad row
                # 32tau+b, col 400tg+g
                nc.sync.dma_start(xw_scr[lst], xwc[:])
                srcv = xw_scr[lst].rearrange("(b tg tau) g -> tau b tg g",
                                             b=4, tg=8)
                padv = xwpad[lst][:].rearrange("(tau q) (tg g) -> tau q tg g",
                                               tau=4, tg=8)
                for tau in range(4):
                    nc.sync.dma_start(padv[tau, 0:4], srcv[tau])

            emit_xw(0)
            emit_xw(1)
            psA_cm.__exit__(None, None, None)
            psB_cm = tc.tile_pool(name="pslstm", bufs=2, space="PSUM")
            psB = psB_cm.__enter__()

            # ========================= LSTM =========================
            hT = []
            for i in range(2):
                hTt = lpool.tile([101, 36], dt.bfloat16, tag=f"hT{i}", bufs=1)
                nc.scalar.dma_start(hTt[:], D['hT_init'][:])
                hT.append(hTt)
            sC = lpool.tile([36, 100], dt.float32, tag="sC", bufs=1)
            nc.gpsimd.memset(sC[:], 0.0)
            # persistent o-gate / cell-tanh tiles (zeros so inactive skew rows
            # transpose to h=0 on the first steps)
            To_t = lpool.tile([36, 100], dt.bfloat16, tag="Tot", bufs=1)
            nc.gpsimd.memset(To_t[:], 0.0)
            th_t = lpool.tile([36, 100], dt.bfloat16, tag="tht16", bufs=1)
            nc.gpsimd.memset(th_t[:], 0.0)
            ToT_s = lpool.tile([100, 36], dt.bfloat16, tag="ToT", bufs=1)
            qsave = lpool.tile([100, 4], dt.bfloat16, tag="qsave", bufs=1)

            whh0v = whh0_s[:].rearrange("p (a g) -> p a g", a=2)
            wih1v = wih1_s[:].rearrange("p (a g) -> p a g", a=2)
            whh1v = whh1_s[:].rearrange("p (a g) -> p a g", a=2)

            if True:
                psl = psB
                pst_o = psl.tile([100, 36], dt.bfloat16, tag="psto", bufs=1)
                pst_th = psl.tile([100, 36], dt.bfloat16, tag="pstth", bufs=1)
                Gs = {}

                def alloc_G(s):
                    """Allocate step-s gate PSUM and pre-accumulate its xw
                    contribution; runs on the PE while the previous step's
                    cell phase occupies Vector/Scalar."""
                    G = psl.tile([36, G4], dt.float32, name="G", tag="gpsum", bufs=4)
                    Gs[s] = G
                    if s <= 63:
                        lo = 0 if s < 32 else 1
                        t0m = s % 32
                        xs = xwpad[lo][:].rearrange("p (tg g) -> p tg g", tg=8)
                        q4 = 32 * (t0m % 4)
                        nc.tensor.matmul(G[0:4, :], i4_s[q4:q4 + 4, :],
                                         xs[q4:q4 + 4, t0m // 4, :],
                                         start=True, stop=False,
                                         tile_position=(q4, 0))

                alloc_G(0)
                for s_ in range(NSTEP):
                    l0_act = s_ <= 63
                    l1_act = 1 <= s_
                    lo = 0 if s_ < 32 else 1
                    l1i = 0 if (s_ - 1) < 32 else 1
                    hp = hT[(s_ - 1) % 2]
                    hn = hT[s_ % 2]
                    p0, p1 = (0, 4) if s_ == 0 else ((32, 36) if s_ == 64 else (0, 36))
                    G = Gs[s_]

                    # h-recurrence matmul leads; l1 follows
                    if l0_act:
                        nc.tensor.matmul(G[0:4, :], hp[0:100, 0:4], whh0v[:, lo, :],
                                         start=False, stop=True, tile_position=(0, 0))
                    if l1_act:
                        nc.tensor.matmul(G[32:36, 0:300], hp[0:100, 0:4],
                                         wih1v[:, l1i, 0:300],
                                         start=True, stop=False, tile_position=(0, 32))
                        nc.tensor.matmul(G[32:36, 0:300], hp[0:101, 32:36],
                                         whh1v[:, l1i, 0:300],
                                         start=False, stop=True, tile_position=(0, 32))
                        nc.tensor.matmul(G[32:36, 300:400], hp[0:100, 0:4],
                                         wih1v[:, l1i, 300:400],
                                         start=True, stop=False, tile_position=(0, 32))
                        nc.tensor.matmul(G[32:36, 300:400], hp[0:101, 32:36],
                                         whh1v[:, l1i, 300:400],
                                         start=False, stop=True, tile_position=(0, 32))

                    # conv-B matmul pieces slot into the cell-phase PE idle
                    npc = 2 if s_ < 14 else 1
                    relus_pending = []
                    if s_ < 32:
                        for _ in range(npc):
                            if pieces:
                                mm, relu = pieces.pop(0)
                                ps = mm()
                                relus_pending.append((relu, ps))

                    T = lpool.tile([36, G4], dt.float32, tag="tanhT")
                    # (f,i,g) on the critical chain; o off-chain into bf16,
                    # transposed early so h is produced directly as hT
                    nc.scalar.activation(T[p0:p1, 0:300], G[p0:p1, 0:300], AF.Tanh,
                                         bias=0.0, scale=0.5)
                    nc.scalar.activation(To_t[p0:p1, :], G[p0:p1, 300:400], AF.Tanh,
                                         bias=0.0, scale=0.5)
                    nc.tensor.transpose(pst_o[:], To_t[:], id36_s[:])
                    u = lpool.tile([36, 100], dt.float32, tag="ut")
                    v = lpool.tile([36, 100], dt.float32, tag="vt")
                    cn = lpool.tile([36, 100], dt.float32, tag="cnt")
                    nc.vector.scalar_tensor_tensor(u[p0:p1, :], T[p0:p1, 0:100], 1.0,
                                                   sC[p0:p1, :], ALU.add, ALU.mult)
                    nc.vector.scalar_tensor_tensor(v[p0:p1, :], T[p0:p1, 100:200], 1.0,
                                                   T[p0:p1, 200:300], ALU.add, ALU.mult)
                    nc.vector.scalar_tensor_tensor(cn[p0:p1, :], v[p0:p1, :], 0.5,
                                                   u[p0:p1, :], ALU.mult, ALU.add)
                    nc.scalar.activation(th_t[p0:p1, :], cn[p0:p1, :], AF.Tanh,
                                         bias=0.0, scale=1.0)
                    nc.vector.tensor_scalar_mul(sC[p0:p1, :], cn[p0:p1, :], 0.5)
                    nc.vector.tensor_copy(ToT_s[:], pst_o[:])
                    for relu, ps in relus_pending:
                        relu(ps)
                    nc.tensor.transpose(pst_th[:], th_t[:], id36_s[:])
                    if s_ + 1 < NSTEP:
                        alloc_G(s_ + 1)
                    # hT = (To^T + 1) * th^T  — straight into the h history
                    nc.vector.scalar_tensor_tensor(hn[0:100, :], ToT_s[:], 1.0,
                                                   pst_th[:], ALU.add, ALU.mult)
                    if s_ == 32:
                        nc.scalar.copy(qsave[:], hn[0:100, 32:36])

            psB_cm.__exit__(None, None, None)

            # ========================= head =========================
            with tc.tile_pool(name="pshead", bufs=1, space="PSUM") as psh:
                hlast = hT[(NSTEP - 1) % 2]
                psf1 = psh.tile([128, 16], dt.float32, tag="psf1")
                fc1v = fc1w_s[:].rearrange("p (a g) -> p a g", a=2)
                for m in range(4):
                    nc.tensor.matmul(psf1[:, 4 * m:4 * m + 4],
                                     fc1v[:, 0, 128 * m:128 * m + 128],
                                     hlast[0:100, 32:36], start=True, stop=False)
                    nc.tensor.matmul(psf1[:, 4 * m:4 * m + 4],
                                     fc1v[:, 1, 128 * m:128 * m + 128],
                                     qsave[:], start=False, stop=True)
                z1 = lpool.tile([128, 16], dt.bfloat16, tag="z1t", bufs=1)
                for m in range(4):
                    # relu on vector (avoids an ACT table swap away from Tanh)
                    nc.vector.scalar_tensor_tensor(
                        z1[:, 4 * m:4 * m + 4], psf1[:, 4 * m:4 * m + 4],
                        fc1b_s[:, m:m + 1], zeros_s[:, 0:4], ALU.add, ALU.max)
                psf2 = psh.tile([4, OUT], dt.float32, tag="psf2")
                fc2v = fc2w_s[:].rearrange("p (a g) -> p a g", a=4)
                for m in range(4):
                    nc.tensor.matmul(psf2[:], z1[:, 4 * m:4 * m + 4], fc2v[:, m, :],
                                     start=(m == 0), stop=False)
                nc.tensor.matmul(psf2[:], ones4_s[:], fc2b_s[:],
                                 start=False, stop=True)
                ot = lpool.tile([4, OUT], dt.float32, tag="outt", bufs=1)
                nc.scalar.copy(ot[:], psf2[:])
                nc.sync.dma_start(out_d[:], ot[:])

    _fix_sync_waits(nc)
    return nc


_NC_CACHE = {}


def _run(inputs):
    inputs = {k: np.asarray(v) for k, v in inputs.items()}
    winp = {k: (np.asarray(v, np.float32) if np.asarray(v).ndim else v)
            for k, v in inputs.items()}
    w = _prep_weights(winp)
    key = 'nc'
    if key not in _NC_CACHE:
        _NC_CACHE[key] = _build_nc()
    nc = _NC_CACHE[key]
    in_maps = [_prep_core_inputs(inputs, w, k) for k in range(N_CORES)]
    res = run_bass_kernel_spmd(nc, in_maps, core_ids=list(range(N_CORES)))
    return res


def kernel(**inputs):
    res = _run(inputs)
    out = np.concatenate([res.results[k]['out'] for k in range(N_CORES)], 0)
    return out.astype(np.float32)
